# revision 11
# baseline (speedup 1.0000x reference)
"""Fused Conv3x3-InstanceNorm-ReLU x2 block for Trainium2, data-parallel over
8 NeuronCores (one batch sample per core).

Per-core pipeline (image [Cin=32 -> Cout=64, 320x320], fp32 in/out):
  pass A: conv1 as row-pair matmuls (K=128 = 4 row-shifted Cin blocks,
          M=128 = 2 output rows x 64 Cout, N=320, fp32r). rhs tiles are
          width-padded to 322 with zero columns so the 3 kw taps are rhs
          slices at offsets 0/1/2 with a full-width PSUM dst (fp32r dst
          restriction). y1 row-pair tiles cached in SBUF (+ tail spilled
          to DRAM), per-channel sum / sum-of-squares accumulated on the fly.
  norm1:  InstanceNorm folded as h = relu(y1 - mu1); the rsqrt scale s1 is
          folded into the conv2 weights (per input channel), so norm+relu is
          a single DVE op per tile.
  pass B: conv2 on normalized row-pair tiles (2 K-blocks of 64 ch), y2 pairs
          cached in SBUF reusing the slots y1 tiles free up, stats on the fly.
  pass C: out = relu((y2 - mu2) * s2) via one ACT op per tile -> DRAM.

Conv biases b1/b2 are accepted but unused: InstanceNorm(affine=False) is
invariant to per-channel constant bias, so they cancel mathematically.
"""
import sys
sys.path.insert(0, '/opt/trn_rl_repo')
import numpy as np
import concourse.bacc as bacc_mod
import concourse.tile as tile
import concourse.mybir as mybir
from concourse.bass_utils import run_bass_kernel_spmd

f32 = mybir.dt.float32
f32r = mybir.dt.float32r
AF = mybir.ActivationFunctionType
OP = mybir.AluOpType

B, CIN, COUT, H, W = 8, 32, 64, 320, 320
WP = W + 2             # padded rhs width
EPS = 1e-5
NP_A = H // 2          # 160 conv1 row pairs (h = 0,2,...,318)
NP_B = H // 2 - 1      # 159 conv2 row pairs (h = 1,3,...,317)
S = 128                # arena slots resident in SBUF
NSP_E = NP_A - S       # spilled y1 pairs
NSP_Y = NP_B - S       # spilled y2 pairs
INV_HW = 1.0 / (H * W)

_CACHE = {}


def _build(repeat=0):
    nc = bacc_mod.Bacc("TRN2", target_bir_lowering=False)
    x_d = nc.dram_tensor("x", [CIN, H, W], f32r, kind="ExternalInput")
    w1_d = nc.dram_tensor("w1", [COUT, CIN, 3, 3], f32r, kind="ExternalInput")
    w2_d = nc.dram_tensor("w2", [COUT, COUT, 3, 3], f32, kind="ExternalInput")
    out_d = nc.dram_tensor("out", [COUT, H, W], f32, kind="ExternalOutput")
    y1sp_d = nc.dram_tensor("y1sp", [max(NSP_E, 1), 128, W], f32r)
    y2sp_d = nc.dram_tensor("y2sp", [max(NSP_Y, 1), 128, W], f32)

    with tile.TileContext(nc) as tc:
        import contextlib
        with contextlib.ExitStack() as ctx:
            arena = ctx.enter_context(tc.tile_pool(name="arena", bufs=S))
            x4p = ctx.enter_context(tc.tile_pool(name="x4p", bufs=3))
            wp = ctx.enter_context(tc.tile_pool(name="wp", bufs=1))
            sp = ctx.enter_context(tc.tile_pool(name="sp", bufs=3))
            op_ = ctx.enter_context(tc.tile_pool(name="op", bufs=3))
            scr = ctx.enter_context(tc.tile_pool(name="scr", bufs=2))
            psp = ctx.enter_context(tc.tile_pool(name="psp", bufs=8, space="PSUM"))


            def body(_iv=None):
                zt = wp.tile([128, WP], f32, tag="zt", name="zt", bufs=1)
                nc.gpsimd.memset(zt[:], 0.0)

                def zero_pads(t):
                    nc.vector.tensor_copy(t[:, 0:1], zt[:, 0:1])
                    nc.vector.tensor_copy(t[:, WP - 1:WP], zt[:, 0:1])

                # ------------- conv1 weights: lhsT [(kh+dr)*32+c, r*64+o] ------
                lw1 = {}
                for v in ("mid", "top", "bot"):
                    for kw in range(3):
                        t = wp.tile([128, 128], f32r, tag=f"lw1{v}{kw}", name=f"lw1{v}{kw}")
                        nc.vector.tensor_copy(t[:], zt[:, 0:128])
                        r0 = range(1, 3) if v == "top" else range(0, 3)
                        for a in r0:
                            nc.sync.dma_start(t[a * 32:(a + 1) * 32, 0:64],
                                              w1_d[:, :, a, kw].rearrange("o c -> c o"))
                        r1 = range(0, 2) if v == "bot" else range(0, 3)
                        for a in r1:
                            nc.sync.dma_start(t[(a + 1) * 32:(a + 2) * 32, 64:128],
                                              w1_d[:, :, a, kw].rearrange("o c -> c o"))
                        lw1[(v, kw)] = t

                s1sum = wp.tile([128, NP_A], f32, tag="s1sum", name="s1sum")
                s1sq = wp.tile([128, NP_A], f32, tag="s1sq", name="s1sq")
                s2sum = wp.tile([128, NP_B + 2], f32, tag="s2sum", name="s2sum")
                s2sq = wp.tile([128, NP_B + 2], f32, tag="s2sq", name="s2sq")
                nc.gpsimd.memset(s2sum[:], 0.0)
                nc.gpsimd.memset(s2sq[:], 0.0)

                # ------------- pass A: conv1 + stats ---------------------------
                E = []
                for k in range(NP_A):
                    h = 2 * k
                    x4 = x4p.tile([128, WP], f32r, tag="x4", name=f"x4_{k}")
                    zero_pads(x4)
                    if k == 0:
                        nc.vector.tensor_copy(x4[0:32, 1:W + 1], zt[0:32, 1:W + 1])
                        for j in range(1, 4):
                            nc.sync.dma_start(x4[j * 32:(j + 1) * 32, 1:W + 1], x_d[:, h - 1 + j, :])
                        lwv = "top"
                    elif k == NP_A - 1:
                        nc.vector.tensor_copy(x4[96:128, 1:W + 1], zt[96:128, 1:W + 1])
                        for j in range(0, 3):
                            nc.sync.dma_start(x4[j * 32:(j + 1) * 32, 1:W + 1], x_d[:, h - 1 + j, :])
                        lwv = "bot"
                    else:
                        for j in range(0, 4):
                            nc.sync.dma_start(x4[j * 32:(j + 1) * 32, 1:W + 1], x_d[:, h - 1 + j, :])
                        lwv = "mid"
                    ps = psp.tile([128, W], f32, tag="pp", name=f"psA{k}")
                    nc.tensor.matmul(ps[:, :], lw1[(lwv, 0)][:], x4[:, 0:W], start=True, stop=False)
                    nc.tensor.matmul(ps[:, :], lw1[(lwv, 1)][:], x4[:, 1:W + 1], start=False, stop=False)
                    nc.tensor.matmul(ps[:, :], lw1[(lwv, 2)][:], x4[:, 2:W + 2], start=False, stop=True)
                    if k < S:
                        e = arena.tile([128, WP], f32r, tag="arena", name=f"E{k}")
                    else:
                        e = sp.tile([128, WP], f32r, tag="espill", name=f"Esp{k}")
                    zero_pads(e)
                    nc.vector.tensor_scalar(e[:, 1:W + 1], ps[:], 0.0, 0.0, OP.add, OP.add,
                                            accum_out=s1sum[:, k:k + 1])
                    sq = scr.tile([128, W], f32, tag="sq", name=f"sqA{k}")
                    nc.scalar.activation(sq[:], ps[:], AF.Square,
                                         accum_out=s1sq[:, k:k + 1])
                    if k >= S:
                        nc.sync.dma_start(y1sp_d[k - S], e[:, 1:W + 1])
                    E.append(e)

                # ------------- stats1 -> -mu1, s1 ------------------------------
                def stats(sumt, sqt, ncols, pfx):
                    red = wp.tile([128, 1], f32, tag=f"{pfx}red", name=f"{pfx}red")
                    redq = wp.tile([128, 1], f32, tag=f"{pfx}redq", name=f"{pfx}redq")
                    nc.vector.tensor_reduce(red[:], sumt[:, 0:ncols], mybir.AxisListType.X, OP.add)
                    nc.vector.tensor_reduce(redq[:], sqt[:, 0:ncols], mybir.AxisListType.X, OP.add)
                    fa = wp.tile([64, 1], f32, tag=f"{pfx}fa", name=f"{pfx}fa")
                    fb = wp.tile([64, 1], f32, tag=f"{pfx}fb", name=f"{pfx}fb")
                    nc.sync.dma_start(fa[:], red[64:128, :])
                    nc.sync.dma_start(fb[:], redq[64:128, :])
                    mu = wp.tile([64, 1], f32, tag=f"{pfx}mu", name=f"{pfx}mu")
                    ex2 = wp.tile([64, 1], f32, tag=f"{pfx}ex2", name=f"{pfx}ex2")
                    nc.vector.tensor_tensor(mu[:], red[0:64, :], fa[:], OP.add)
                    nc.vector.tensor_tensor(ex2[:], redq[0:64, :], fb[:], OP.add)
                    nc.vector.tensor_scalar(mu[:], mu[:], INV_HW, None, OP.mult)
                    nc.vector.tensor_scalar(ex2[:], ex2[:], INV_HW, None, OP.mult)
                    var = wp.tile([64, 1], f32, tag=f"{pfx}var", name=f"{pfx}var")
                    nc.vector.tensor_tensor(var[:], mu[:], mu[:], OP.mult)
                    nc.vector.tensor_tensor(var[:], ex2[:], var[:], OP.subtract)
                    nc.vector.tensor_scalar(var[:], var[:], EPS, None, OP.add)
                    sd = wp.tile([64, 1], f32, tag=f"{pfx}sd", name=f"{pfx}sd")
                    nc.scalar.activation(sd[:], var[:], AF.Sqrt)
                    s = wp.tile([64, 1], f32, tag=f"{pfx}s", name=f"{pfx}s")
                    nc.vector.reciprocal(s[:], sd[:])
                    return mu, s

                mu1, s1 = stats(s1sum, s1sq, NP_A, "st1")
                nmu1v = wp.tile([128, 1], f32, tag="nmu1v", name="nmu1v")
                negmu1 = wp.tile([64, 1], f32, tag="negmu1", name="negmu1")
                nc.vector.tensor_scalar(negmu1[:], mu1[:], -1.0, None, OP.mult)
                nc.sync.dma_start(nmu1v[0:64, :], negmu1[:])
                nc.sync.dma_start(nmu1v[64:128, :], negmu1[:])
                s1v = wp.tile([128, 1], f32, tag="s1v", name="s1v")
                nc.sync.dma_start(s1v[0:64, :], s1[:])
                nc.sync.dma_start(s1v[64:128, :], s1[:])

                # ------------- conv2 weights (scaled by s1) --------------------
                def w2k(kh, kw):
                    return w2_d[:, :, kh, kw].rearrange("o c -> c o")

                lw2 = {}
                for kw in range(3):
                    stA = x4p.tile([128, 128], f32, tag="wstg", name=f"stA{kw}")
                    nc.gpsimd.memset(stA[:], 0.0)
                    nc.sync.dma_start(stA[0:64, 0:64], w2k(0, kw))
                    nc.sync.dma_start(stA[64:128, 0:64], w2k(1, kw))
                    nc.sync.dma_start(stA[64:128, 64:128], w2k(0, kw))
                    tA = wp.tile([128, 128], f32r, tag=f"lw2A{kw}", name=f"lw2A{kw}")
                    nc.vector.tensor_scalar(tA[:], stA[:], s1v[:, 0:1], None, OP.mult)
                    lw2[("A", kw)] = tA
                    stB = x4p.tile([128, 128], f32, tag="wstg", name=f"stB{kw}")
                    nc.gpsimd.memset(stB[:], 0.0)
                    nc.sync.dma_start(stB[0:64, 0:64], w2k(2, kw))
                    nc.sync.dma_start(stB[0:64, 64:128], w2k(1, kw))
                    nc.sync.dma_start(stB[64:128, 64:128], w2k(2, kw))
                    tB = wp.tile([128, 128], f32r, tag=f"lw2B{kw}", name=f"lw2B{kw}")
                    nc.vector.tensor_scalar(tB[:], stB[:], s1v[:, 0:1], None, OP.mult)
                    lw2[("B", kw)] = tB
                    st0 = x4p.tile([128, 64], f32, tag="wstg0", name=f"st0{kw}")
                    nc.sync.dma_start(st0[0:64, :], w2k(1, kw))
                    nc.sync.dma_start(st0[64:128, :], w2k(2, kw))
                    t0 = wp.tile([128, 64], f32r, tag=f"lw2S0{kw}", name=f"lw2S0{kw}")
                    nc.vector.tensor_scalar(t0[:], st0[:], s1v[:, 0:1], None, OP.mult)
                    lw2[("S0", kw)] = t0
                    st9 = x4p.tile([128, 64], f32, tag="wstg0", name=f"st9{kw}")
                    nc.sync.dma_start(st9[0:64, :], w2k(0, kw))
                    nc.sync.dma_start(st9[64:128, :], w2k(1, kw))
                    t9 = wp.tile([128, 64], f32r, tag=f"lw2S9{kw}", name=f"lw2S9{kw}")
                    nc.vector.tensor_scalar(t9[:], st9[:], s1v[:, 0:1], None, OP.mult)
                    lw2[("S9", kw)] = t9

                # ------------- normalize resident y1: e = relu(e - mu1) --------
                for k in range(S):
                    nc.vector.tensor_scalar(E[k][:, 1:W + 1], E[k][:, 1:W + 1],
                                            nmu1v[:, 0:1], 0.0, OP.add, OP.max)

                eloaded = {}

                def getE(i):
                    if i < S:
                        return E[i]
                    if i not in eloaded:
                        t = sp.tile([128, WP], f32r, tag="eload", name=f"Eld{i}")
                        zero_pads(t)
                        nc.sync.dma_start(t[:, 1:W + 1], y1sp_d[i - S])
                        nc.vector.tensor_scalar(t[:, 1:W + 1], t[:, 1:W + 1],
                                                nmu1v[:, 0:1], 0.0, OP.add, OP.max)
                        eloaded[i] = t
                    return eloaded[i]

                # ------------- pass B: conv2 + stats ---------------------------
                # single row 0: taps kh=1,2 from rows 0,1 (E[0])
                ps0 = psp.tile([64, W], f32, tag="pp", name="psS0")
                w_ = [lw2[("S0", kw)] for kw in range(3)]
                nc.tensor.matmul(ps0[:, :], w_[0][:], E[0][:, 0:W], start=True, stop=False)
                nc.tensor.matmul(ps0[:, :], w_[1][:], E[0][:, 1:W + 1], start=False, stop=False)
                nc.tensor.matmul(ps0[:, :], w_[2][:], E[0][:, 2:W + 2], start=False, stop=True)
                y0 = wp.tile([64, W], f32, tag="ys0", name="ys0")
                nc.vector.tensor_scalar(y0[:], ps0[:], 0.0, 0.0, OP.add, OP.add,
                                        accum_out=s2sum[0:64, NP_B:NP_B + 1])
                sq0 = scr.tile([64, W], f32, tag="sq", name="sqS0")
                nc.scalar.activation(sq0[:], ps0[:], AF.Square,
                                     accum_out=s2sq[0:64, NP_B:NP_B + 1])

                Y = []
                for kb in range(NP_B):
                    eA = getE(kb)
                    eB = getE(kb + 1)
                    ps = psp.tile([128, W], f32, tag="pp", name=f"psB{kb}")
                    wA = [lw2[("A", kw)] for kw in range(3)]
                    wB = [lw2[("B", kw)] for kw in range(3)]
                    nc.tensor.matmul(ps[:, :], wA[0][:], eA[:, 0:W], start=True, stop=False)
                    nc.tensor.matmul(ps[:, :], wA[1][:], eA[:, 1:W + 1], start=False, stop=False)
                    nc.tensor.matmul(ps[:, :], wA[2][:], eA[:, 2:W + 2], start=False, stop=False)
                    nc.tensor.matmul(ps[:, :], wB[0][:], eB[:, 0:W], start=False, stop=False)
                    nc.tensor.matmul(ps[:, :], wB[1][:], eB[:, 1:W + 1], start=False, stop=False)
                    nc.tensor.matmul(ps[:, :], wB[2][:], eB[:, 2:W + 2], start=False, stop=True)
                    if kb < S:
                        y = arena.tile([128, W], f32, tag="arena", name=f"Y{kb}")
                    else:
                        y = sp.tile([128, W], f32, tag="yspill", name=f"Ysp{kb}")
                    nc.vector.tensor_scalar(y[:], ps[:], 0.0, 0.0, OP.add, OP.add,
                                            accum_out=s2sum[:, kb:kb + 1])
                    sqb = scr.tile([128, W], f32, tag="sq", name=f"sqB{kb}")
                    nc.scalar.activation(sqb[:], ps[:], AF.Square,
                                         accum_out=s2sq[:, kb:kb + 1])
                    if kb >= S:
                        nc.sync.dma_start(y2sp_d[kb - S], y[:])
                    Y.append(y)

                # single row 319: taps kh=0,1 from rows 318,319 (E[159])
                e9 = getE(NP_A - 1)
                ps9 = psp.tile([64, W], f32, tag="pp", name="psS9")
                w_ = [lw2[("S9", kw)] for kw in range(3)]
                nc.tensor.matmul(ps9[:, :], w_[0][:], e9[:, 0:W], start=True, stop=False)
                nc.tensor.matmul(ps9[:, :], w_[1][:], e9[:, 1:W + 1], start=False, stop=False)
                nc.tensor.matmul(ps9[:, :], w_[2][:], e9[:, 2:W + 2], start=False, stop=True)
                y9 = wp.tile([64, W], f32, tag="ys9", name="ys9")
                nc.vector.tensor_scalar(y9[:], ps9[:], 0.0, 0.0, OP.add, OP.add,
                                        accum_out=s2sum[0:64, NP_B + 1:NP_B + 2])
                sq9 = scr.tile([64, W], f32, tag="sq", name="sqS9")
                nc.scalar.activation(sq9[:], ps9[:], AF.Square,
                                     accum_out=s2sq[0:64, NP_B + 1:NP_B + 2])

                # ------------- stats2 -> s2, t2 = -mu2*s2 ----------------------
                mu2, s2 = stats(s2sum, s2sq, NP_B + 2, "st2")
                t2 = wp.tile([64, 1], f32, tag="t2", name="t2")
                nc.vector.tensor_tensor(t2[:], mu2[:], s2[:], OP.mult)
                nc.vector.tensor_scalar(t2[:], t2[:], -1.0, None, OP.mult)
                s2v = wp.tile([128, 1], f32, tag="s2v", name="s2v")
                t2v = wp.tile([128, 1], f32, tag="t2v", name="t2v")
                nc.sync.dma_start(s2v[0:64, :], s2[:])
                nc.sync.dma_start(s2v[64:128, :], s2[:])
                nc.sync.dma_start(t2v[0:64, :], t2[:])
                nc.sync.dma_start(t2v[64:128, :], t2[:])

                # ------------- pass C: out = relu(y2*s2 + t2) ------------------
                co0 = op_.tile([64, W], f32, tag="co", name="co0")
                nc.scalar.activation(co0[:], y0[:], AF.Relu,
                                     bias=t2v[0:64, 0:1], scale=s2v[0:64, 0:1])
                nc.sync.dma_start(out_d[:, 0, :], co0[:])
                for kb in range(NP_B):
                    h = 2 * kb + 1
                    if kb < S:
                        ysrc = Y[kb]
                    else:
                        ysrc = sp.tile([128, W], f32, tag="yload", name=f"Yld{kb}")
                        nc.sync.dma_start(ysrc[:], y2sp_d[kb - S])
                    co = op_.tile([128, W], f32, tag="co", name=f"co{kb}")
                    nc.scalar.activation(co[:], ysrc[:], AF.Relu,
                                         bias=t2v[:, 0:1], scale=s2v[:, 0:1])
                    nc.sync.dma_start(out_d[:, h, :], co[0:64, :])
                    nc.sync.dma_start(out_d[:, h + 1, :], co[64:128, :])
                co9 = op_.tile([64, W], f32, tag="co", name="co9")
                nc.scalar.activation(co9[:], y9[:], AF.Relu,
                                     bias=t2v[0:64, 0:1], scale=s2v[0:64, 0:1])
                nc.sync.dma_start(out_d[:, H - 1, :], co9[:])

            if repeat:
                with tc.For_i(0, repeat, 1, hint_engines=(mybir.EngineType.PE,)):
                    body()
            else:
                body()

    nc.finalize()
    return nc


def _get_nc(repeat=0):
    key = ("nc", repeat)
    if key not in _CACHE:
        _CACHE[key] = _build(repeat)
    return _CACHE[key]


def kernel(x, w1, b1=None, w2=None, b2=None, **kw):
    x = np.ascontiguousarray(np.asarray(x, dtype=np.float32))
    w1 = np.ascontiguousarray(np.asarray(w1, dtype=np.float32))
    w2 = np.ascontiguousarray(np.asarray(w2, dtype=np.float32))
    nc = _get_nc()
    in_maps = [{"x": x[i], "w1": w1, "w2": w2} for i in range(B)]
    res = run_bass_kernel_spmd(nc, in_maps, list(range(B)), trace=False)
    return np.stack([res.results[i]["out"] for i in range(B)], axis=0)


# revision 13
# speedup vs baseline: 2.2616x; 2.2616x over previous
"""Fused Conv3x3-InstanceNorm-ReLU x2 block for Trainium2, data-parallel over
8 NeuronCores (one batch sample per core).

Per-core pipeline (image [Cin=32 -> Cout=64, 320x320], fp32 in/out):
  pass A: conv1 as row-pair matmuls (K=128 = 4 row-shifted Cin blocks,
          M=128 = 2 output rows x 64 Cout, N=320, fp32r). rhs tiles are
          width-padded to 322 with zero columns so the 3 kw taps are rhs
          slices at offsets 0/1/2 with a full-width PSUM dst (fp32r dst
          restriction). y1 row-pair tiles cached in SBUF (+ tail spilled
          to DRAM), per-channel sum / sum-of-squares accumulated on the fly.
  norm1:  InstanceNorm folded as h = relu(y1 - mu1); the rsqrt scale s1 is
          folded into the conv2 weights (per input channel), so norm+relu is
          a single DVE op per tile.
  pass B: conv2 on normalized row-pair tiles (2 K-blocks of 64 ch), y2 pairs
          cached in SBUF reusing the slots y1 tiles free up, stats on the fly.
  pass C: out = relu((y2 - mu2) * s2) via one ACT op per tile -> DRAM.

Conv biases b1/b2 are accepted but unused: InstanceNorm(affine=False) is
invariant to per-channel constant bias, so they cancel mathematically.
"""
import sys
sys.path.insert(0, '/opt/trn_rl_repo')
import numpy as np
import concourse.bacc as bacc_mod
import concourse.tile as tile
import concourse.mybir as mybir
from concourse.bass_utils import run_bass_kernel_spmd

f32 = mybir.dt.float32
f32r = mybir.dt.float32r
AF = mybir.ActivationFunctionType
OP = mybir.AluOpType

B, CIN, COUT, H, W = 8, 32, 64, 320, 320
WP = W + 2             # padded rhs width
EPS = 1e-5
NP_A = H // 2          # 160 conv1 row pairs (h = 0,2,...,318)
NP_B = H // 2 - 1      # 159 conv2 row pairs (h = 1,3,...,317)
S = 128                # arena slots resident in SBUF
NSP_E = NP_A - S       # spilled y1 pairs
NSP_Y = NP_B - S       # spilled y2 pairs
INV_HW = 1.0 / (H * W)

_CACHE = {}


def _build(repeat=0):
    nc = bacc_mod.Bacc("TRN2", target_bir_lowering=False)
    x_d = nc.dram_tensor("x", [CIN, H, W], f32r, kind="ExternalInput")
    w1_d = nc.dram_tensor("w1", [COUT, CIN, 3, 3], f32r, kind="ExternalInput")
    w2_d = nc.dram_tensor("w2", [COUT, COUT, 3, 3], f32, kind="ExternalInput")
    out_d = nc.dram_tensor("out", [COUT, H, W], f32, kind="ExternalOutput")
    y1sp_d = nc.dram_tensor("y1sp", [max(NSP_E, 1), 128, W], f32r)
    y2sp_d = nc.dram_tensor("y2sp", [max(NSP_Y, 1), 128, W], f32)

    with tile.TileContext(nc) as tc:
        import contextlib
        with contextlib.ExitStack() as ctx:
            arena = ctx.enter_context(tc.tile_pool(name="arena", bufs=S))
            x4p = ctx.enter_context(tc.tile_pool(name="x4p", bufs=3))
            wp = ctx.enter_context(tc.tile_pool(name="wp", bufs=1))
            sp = ctx.enter_context(tc.tile_pool(name="sp", bufs=3))
            op_ = ctx.enter_context(tc.tile_pool(name="op", bufs=3))
            scr = ctx.enter_context(tc.tile_pool(name="scr", bufs=2))
            psp = ctx.enter_context(tc.tile_pool(name="psp", bufs=8, space="PSUM"))


            def body(_iv=None):
                zt = wp.tile([128, WP], f32, tag="zt", name="zt", bufs=1)
                nc.gpsimd.memset(zt[:], 0.0)

                def zero_pads(t):
                    nc.vector.tensor_copy(t[:, 0:1], zt[:, 0:1])
                    nc.vector.tensor_copy(t[:, WP - 1:WP], zt[:, 0:1])

                # ------------- conv1 weights: lhsT [(kh+dr)*32+c, r*64+o] ------
                lw1 = {}
                for v in ("mid", "top", "bot"):
                    for kw in range(3):
                        t = wp.tile([128, 128], f32r, tag=f"lw1{v}{kw}", name=f"lw1{v}{kw}")
                        nc.vector.tensor_copy(t[:], zt[:, 0:128])
                        r0 = range(1, 3) if v == "top" else range(0, 3)
                        for a in r0:
                            nc.sync.dma_start(t[a * 32:(a + 1) * 32, 0:64],
                                              w1_d[:, :, a, kw].rearrange("o c -> c o"))
                        r1 = range(0, 2) if v == "bot" else range(0, 3)
                        for a in r1:
                            nc.sync.dma_start(t[(a + 1) * 32:(a + 2) * 32, 64:128],
                                              w1_d[:, :, a, kw].rearrange("o c -> c o"))
                        lw1[(v, kw)] = t

                s1sum = wp.tile([128, NP_A], f32, tag="s1sum", name="s1sum")
                s1sq = wp.tile([128, NP_A], f32, tag="s1sq", name="s1sq")
                s2sum = wp.tile([128, NP_B + 2], f32, tag="s2sum", name="s2sum")
                s2sq = wp.tile([128, NP_B + 2], f32, tag="s2sq", name="s2sq")
                nc.gpsimd.memset(s2sum[:], 0.0)
                nc.gpsimd.memset(s2sq[:], 0.0)

                # ------------- pass A: conv1 + stats ---------------------------
                E = []
                for k in range(NP_A):
                    h = 2 * k
                    x4 = x4p.tile([128, WP], f32r, tag="x4", name=f"x4_{k}")
                    zero_pads(x4)
                    if k == 0:
                        nc.vector.tensor_copy(x4[0:32, 1:W + 1], zt[0:32, 1:W + 1])
                        nc.sync.dma_start(x4[32:128, 1:W + 1],
                                          x_d[:, 0:3, :].rearrange("c j w -> j c w"))
                        lwv = "top"
                    elif k == NP_A - 1:
                        nc.vector.tensor_copy(x4[96:128, 1:W + 1], zt[96:128, 1:W + 1])
                        nc.sync.dma_start(x4[0:96, 1:W + 1],
                                          x_d[:, H - 3:H, :].rearrange("c j w -> j c w"))
                        lwv = "bot"
                    else:
                        nc.sync.dma_start(x4[:, 1:W + 1],
                                          x_d[:, h - 1:h + 3, :].rearrange("c j w -> j c w"))
                        lwv = "mid"
                    ps = psp.tile([128, W], f32, tag="pp", name=f"psA{k}")
                    nc.tensor.matmul(ps[:, :], lw1[(lwv, 0)][:], x4[:, 0:W], start=True, stop=False)
                    nc.tensor.matmul(ps[:, :], lw1[(lwv, 1)][:], x4[:, 1:W + 1], start=False, stop=False)
                    nc.tensor.matmul(ps[:, :], lw1[(lwv, 2)][:], x4[:, 2:W + 2], start=False, stop=True)
                    if k < S:
                        e = arena.tile([128, WP], f32r, tag="arena", name=f"E{k}")
                    else:
                        e = sp.tile([128, WP], f32r, tag="espill", name=f"Esp{k}")
                    zero_pads(e)
                    nc.vector.tensor_scalar(e[:, 1:W + 1], ps[:], 0.0, 0.0, OP.add, OP.add,
                                            accum_out=s1sum[:, k:k + 1])
                    sq = scr.tile([128, W], f32, tag="sq", name=f"sqA{k}")
                    nc.scalar.activation(sq[:], ps[:], AF.Square,
                                         accum_out=s1sq[:, k:k + 1])
                    if k >= S:
                        nc.sync.dma_start(y1sp_d[k - S], e[:, 1:W + 1])
                    E.append(e)

                # ------------- stats1 -> -mu1, s1 ------------------------------
                def stats(sumt, sqt, ncols, pfx):
                    red = wp.tile([128, 1], f32, tag=f"{pfx}red", name=f"{pfx}red")
                    redq = wp.tile([128, 1], f32, tag=f"{pfx}redq", name=f"{pfx}redq")
                    nc.vector.tensor_reduce(red[:], sumt[:, 0:ncols], mybir.AxisListType.X, OP.add)
                    nc.vector.tensor_reduce(redq[:], sqt[:, 0:ncols], mybir.AxisListType.X, OP.add)
                    fa = wp.tile([64, 1], f32, tag=f"{pfx}fa", name=f"{pfx}fa")
                    fb = wp.tile([64, 1], f32, tag=f"{pfx}fb", name=f"{pfx}fb")
                    nc.sync.dma_start(fa[:], red[64:128, :])
                    nc.sync.dma_start(fb[:], redq[64:128, :])
                    mu = wp.tile([64, 1], f32, tag=f"{pfx}mu", name=f"{pfx}mu")
                    ex2 = wp.tile([64, 1], f32, tag=f"{pfx}ex2", name=f"{pfx}ex2")
                    nc.vector.tensor_tensor(mu[:], red[0:64, :], fa[:], OP.add)
                    nc.vector.tensor_tensor(ex2[:], redq[0:64, :], fb[:], OP.add)
                    nc.vector.tensor_scalar(mu[:], mu[:], INV_HW, None, OP.mult)
                    nc.vector.tensor_scalar(ex2[:], ex2[:], INV_HW, None, OP.mult)
                    var = wp.tile([64, 1], f32, tag=f"{pfx}var", name=f"{pfx}var")
                    nc.vector.tensor_tensor(var[:], mu[:], mu[:], OP.mult)
                    nc.vector.tensor_tensor(var[:], ex2[:], var[:], OP.subtract)
                    nc.vector.tensor_scalar(var[:], var[:], EPS, None, OP.add)
                    sd = wp.tile([64, 1], f32, tag=f"{pfx}sd", name=f"{pfx}sd")
                    nc.scalar.activation(sd[:], var[:], AF.Sqrt)
                    s = wp.tile([64, 1], f32, tag=f"{pfx}s", name=f"{pfx}s")
                    nc.vector.reciprocal(s[:], sd[:])
                    return mu, s

                mu1, s1 = stats(s1sum, s1sq, NP_A, "st1")
                nmu1v = wp.tile([128, 1], f32, tag="nmu1v", name="nmu1v")
                negmu1 = wp.tile([64, 1], f32, tag="negmu1", name="negmu1")
                nc.vector.tensor_scalar(negmu1[:], mu1[:], -1.0, None, OP.mult)
                nc.sync.dma_start(nmu1v[0:64, :], negmu1[:])
                nc.sync.dma_start(nmu1v[64:128, :], negmu1[:])
                s1v = wp.tile([128, 1], f32, tag="s1v", name="s1v")
                nc.sync.dma_start(s1v[0:64, :], s1[:])
                nc.sync.dma_start(s1v[64:128, :], s1[:])

                # ------------- conv2 weights (scaled by s1) --------------------
                def w2k(kh, kw):
                    return w2_d[:, :, kh, kw].rearrange("o c -> c o")

                lw2 = {}
                for kw in range(3):
                    stA = x4p.tile([128, 128], f32, tag="wstg", name=f"stA{kw}")
                    nc.gpsimd.memset(stA[:], 0.0)
                    nc.sync.dma_start(stA[0:64, 0:64], w2k(0, kw))
                    nc.sync.dma_start(stA[64:128, 0:64], w2k(1, kw))
                    nc.sync.dma_start(stA[64:128, 64:128], w2k(0, kw))
                    tA = wp.tile([128, 128], f32r, tag=f"lw2A{kw}", name=f"lw2A{kw}")
                    nc.vector.tensor_scalar(tA[:], stA[:], s1v[:, 0:1], None, OP.mult)
                    lw2[("A", kw)] = tA
                    stB = x4p.tile([128, 128], f32, tag="wstg", name=f"stB{kw}")
                    nc.gpsimd.memset(stB[:], 0.0)
                    nc.sync.dma_start(stB[0:64, 0:64], w2k(2, kw))
                    nc.sync.dma_start(stB[0:64, 64:128], w2k(1, kw))
                    nc.sync.dma_start(stB[64:128, 64:128], w2k(2, kw))
                    tB = wp.tile([128, 128], f32r, tag=f"lw2B{kw}", name=f"lw2B{kw}")
                    nc.vector.tensor_scalar(tB[:], stB[:], s1v[:, 0:1], None, OP.mult)
                    lw2[("B", kw)] = tB
                    st0 = x4p.tile([128, 64], f32, tag="wstg0", name=f"st0{kw}")
                    nc.sync.dma_start(st0[0:64, :], w2k(1, kw))
                    nc.sync.dma_start(st0[64:128, :], w2k(2, kw))
                    t0 = wp.tile([128, 64], f32r, tag=f"lw2S0{kw}", name=f"lw2S0{kw}")
                    nc.vector.tensor_scalar(t0[:], st0[:], s1v[:, 0:1], None, OP.mult)
                    lw2[("S0", kw)] = t0
                    st9 = x4p.tile([128, 64], f32, tag="wstg0", name=f"st9{kw}")
                    nc.sync.dma_start(st9[0:64, :], w2k(0, kw))
                    nc.sync.dma_start(st9[64:128, :], w2k(1, kw))
                    t9 = wp.tile([128, 64], f32r, tag=f"lw2S9{kw}", name=f"lw2S9{kw}")
                    nc.vector.tensor_scalar(t9[:], st9[:], s1v[:, 0:1], None, OP.mult)
                    lw2[("S9", kw)] = t9

                # ------------- normalize resident y1: e = relu(e - mu1) --------
                for k in range(S):
                    nc.vector.tensor_scalar(E[k][:, 1:W + 1], E[k][:, 1:W + 1],
                                            nmu1v[:, 0:1], 0.0, OP.add, OP.max)

                eloaded = {}

                def getE(i):
                    if i < S:
                        return E[i]
                    if i not in eloaded:
                        t = sp.tile([128, WP], f32r, tag="eload", name=f"Eld{i}")
                        zero_pads(t)
                        nc.sync.dma_start(t[:, 1:W + 1], y1sp_d[i - S])
                        nc.vector.tensor_scalar(t[:, 1:W + 1], t[:, 1:W + 1],
                                                nmu1v[:, 0:1], 0.0, OP.add, OP.max)
                        eloaded[i] = t
                    return eloaded[i]

                # ------------- pass B: conv2 + stats ---------------------------
                # single row 0: taps kh=1,2 from rows 0,1 (E[0])
                ps0 = psp.tile([64, W], f32, tag="pp", name="psS0")
                w_ = [lw2[("S0", kw)] for kw in range(3)]
                nc.tensor.matmul(ps0[:, :], w_[0][:], E[0][:, 0:W], start=True, stop=False)
                nc.tensor.matmul(ps0[:, :], w_[1][:], E[0][:, 1:W + 1], start=False, stop=False)
                nc.tensor.matmul(ps0[:, :], w_[2][:], E[0][:, 2:W + 2], start=False, stop=True)
                y0 = wp.tile([64, W], f32, tag="ys0", name="ys0")
                nc.vector.tensor_scalar(y0[:], ps0[:], 0.0, 0.0, OP.add, OP.add,
                                        accum_out=s2sum[0:64, NP_B:NP_B + 1])
                sq0 = scr.tile([64, W], f32, tag="sq", name="sqS0")
                nc.scalar.activation(sq0[:], ps0[:], AF.Square,
                                     accum_out=s2sq[0:64, NP_B:NP_B + 1])

                Y = []
                for kb in range(NP_B):
                    eA = getE(kb)
                    eB = getE(kb + 1)
                    ps = psp.tile([128, W], f32, tag="pp", name=f"psB{kb}")
                    wA = [lw2[("A", kw)] for kw in range(3)]
                    wB = [lw2[("B", kw)] for kw in range(3)]
                    nc.tensor.matmul(ps[:, :], wA[0][:], eA[:, 0:W], start=True, stop=False)
                    nc.tensor.matmul(ps[:, :], wA[1][:], eA[:, 1:W + 1], start=False, stop=False)
                    nc.tensor.matmul(ps[:, :], wA[2][:], eA[:, 2:W + 2], start=False, stop=False)
                    nc.tensor.matmul(ps[:, :], wB[0][:], eB[:, 0:W], start=False, stop=False)
                    nc.tensor.matmul(ps[:, :], wB[1][:], eB[:, 1:W + 1], start=False, stop=False)
                    nc.tensor.matmul(ps[:, :], wB[2][:], eB[:, 2:W + 2], start=False, stop=True)
                    if kb < S:
                        y = arena.tile([128, W], f32, tag="arena", name=f"Y{kb}")
                    else:
                        y = sp.tile([128, W], f32, tag="yspill", name=f"Ysp{kb}")
                    nc.vector.tensor_scalar(y[:], ps[:], 0.0, 0.0, OP.add, OP.add,
                                            accum_out=s2sum[:, kb:kb + 1])
                    sqb = scr.tile([128, W], f32, tag="sq", name=f"sqB{kb}")
                    nc.scalar.activation(sqb[:], ps[:], AF.Square,
                                         accum_out=s2sq[:, kb:kb + 1])
                    if kb >= S:
                        nc.sync.dma_start(y2sp_d[kb - S], y[:])
                    Y.append(y)

                # single row 319: taps kh=0,1 from rows 318,319 (E[159])
                e9 = getE(NP_A - 1)
                ps9 = psp.tile([64, W], f32, tag="pp", name="psS9")
                w_ = [lw2[("S9", kw)] for kw in range(3)]
                nc.tensor.matmul(ps9[:, :], w_[0][:], e9[:, 0:W], start=True, stop=False)
                nc.tensor.matmul(ps9[:, :], w_[1][:], e9[:, 1:W + 1], start=False, stop=False)
                nc.tensor.matmul(ps9[:, :], w_[2][:], e9[:, 2:W + 2], start=False, stop=True)
                y9 = wp.tile([64, W], f32, tag="ys9", name="ys9")
                nc.vector.tensor_scalar(y9[:], ps9[:], 0.0, 0.0, OP.add, OP.add,
                                        accum_out=s2sum[0:64, NP_B + 1:NP_B + 2])
                sq9 = scr.tile([64, W], f32, tag="sq", name="sqS9")
                nc.scalar.activation(sq9[:], ps9[:], AF.Square,
                                     accum_out=s2sq[0:64, NP_B + 1:NP_B + 2])

                # ------------- stats2 -> s2, t2 = -mu2*s2 ----------------------
                mu2, s2 = stats(s2sum, s2sq, NP_B + 2, "st2")
                t2 = wp.tile([64, 1], f32, tag="t2", name="t2")
                nc.vector.tensor_tensor(t2[:], mu2[:], s2[:], OP.mult)
                nc.vector.tensor_scalar(t2[:], t2[:], -1.0, None, OP.mult)
                s2v = wp.tile([128, 1], f32, tag="s2v", name="s2v")
                t2v = wp.tile([128, 1], f32, tag="t2v", name="t2v")
                nc.sync.dma_start(s2v[0:64, :], s2[:])
                nc.sync.dma_start(s2v[64:128, :], s2[:])
                nc.sync.dma_start(t2v[0:64, :], t2[:])
                nc.sync.dma_start(t2v[64:128, :], t2[:])

                # ------------- pass C: out = relu(y2*s2 + t2) ------------------
                co0 = op_.tile([64, W], f32, tag="co", name="co0")
                nc.scalar.activation(co0[:], y0[:], AF.Relu,
                                     bias=t2v[0:64, 0:1], scale=s2v[0:64, 0:1])
                nc.scalar.dma_start(out_d[:, 0, :], co0[:])
                for kb in range(NP_B):
                    h = 2 * kb + 1
                    if kb < S:
                        ysrc = Y[kb]
                    else:
                        ysrc = sp.tile([128, W], f32, tag="yload", name=f"Yld{kb}")
                        nc.sync.dma_start(ysrc[:], y2sp_d[kb - S])
                    co = op_.tile([128, W], f32, tag="co", name=f"co{kb}")
                    nc.scalar.activation(co[:], ysrc[:], AF.Relu,
                                         bias=t2v[:, 0:1], scale=s2v[:, 0:1])
                    nc.scalar.dma_start(out_d[:, h:h + 2, :].rearrange("o r w -> r o w"), co[:])
                co9 = op_.tile([64, W], f32, tag="co", name="co9")
                nc.scalar.activation(co9[:], y9[:], AF.Relu,
                                     bias=t2v[0:64, 0:1], scale=s2v[0:64, 0:1])
                nc.scalar.dma_start(out_d[:, H - 1, :], co9[:])

            if repeat:
                with tc.For_i(0, repeat, 1, hint_engines=(mybir.EngineType.PE,)):
                    body()
            else:
                body()

    nc.finalize()
    return nc


def _get_nc(repeat=0):
    key = ("nc", repeat)
    if key not in _CACHE:
        _CACHE[key] = _build(repeat)
    return _CACHE[key]


def kernel(x, w1, b1=None, w2=None, b2=None, **kw):
    x = np.ascontiguousarray(np.asarray(x, dtype=np.float32))
    w1 = np.ascontiguousarray(np.asarray(w1, dtype=np.float32))
    w2 = np.ascontiguousarray(np.asarray(w2, dtype=np.float32))
    nc = _get_nc()
    in_maps = [{"x": x[i], "w1": w1, "w2": w2} for i in range(B)]
    res = run_bass_kernel_spmd(nc, in_maps, list(range(B)), trace=False)
    return np.stack([res.results[i]["out"] for i in range(B)], axis=0)
